# revision 27
# baseline (speedup 1.0000x reference)
"""Trainium2 Bass kernel for the DimeNet-style directed-message block.

Reference computation (W = n_angles, E = n_edges, D = 128, A = 49, J = 8):
    m_kj     = m_ji[kj_idx]                          # [W, D]
    transf_m = silu(m_kj @ W_nbr + b_nbr)            # [W, D]
    transf_e = e_rbf[kj_idx] @ W_e                   # [W, D]
    m_and_e  = transf_m * transf_e                   # [W, D]
    transf_a = a_sbf @ W_a                           # [W, J]
    out[w,i] = sum_{j,l} transf_a[w,j] m_and_e[w,l] final_w[i,j,l]
    final    = segment_sum(out, kj_idx, E)           # [E, D]

Algebraic refactor: every per-angle factor except transf_a depends on the
angle only through kj_idx, so the segment sum commutes through the bilinear
form:
    me       = silu(m_ji @ W_nbr + b) * (e_rbf @ W_e)        # [E, D]
    S        = segment_sum(a_sbf @ W_a, kj_idx, E)           # [E, J]
    final[e] = sum_j S[e,j] * (me[e] @ final_w[:,j,:].T)     # [E, D]

S is computed without any scatter: the host assigns each edge a slot,
SORTED BY ANGLE COUNT (descending), and bins each edge's angles into rank
slots (rank r = r-th angle of its edge).  Rank pass r of a 512-edge chunk
streams a_sbf^T through the PE aligned by slot; PSUM accumulation over the
rank passes IS the segment sum.  A rank PAIR occupies one 128-partition
block (even rank rows 0:49, odd rank rows 64:113); the lhsT holds W_a twice
at the same offsets so a single matmul contracts and sums both ranks.
Because slots are count-sorted, each chunk streams only ceil(max_count/2)
rank pairs -- no overflow levels, no scatter, no DRAM accumulator.

The apply stage computes y = me_g^T @ t2 per 128-edge group with t2 in
j-OUTER order (col = j*128+i), scales by S, and reduces over j with
per-chunk batched adds (all tree levels i-packed so bf16 adds hit the DVE
2x mode).  The scale multiplies rotate across engines to balance load
(D: DVE broadcast-multiply from PSUM, C: 8 per-j scalar-engine scaled
copies, B: scalar-engine evacuation + gpsimd multiply); the adds are
software-pipelined one chunk behind the multiplies so cross-engine
stragglers do not stall the DVE queue.  L2/L3 of the reduction run on
gpsimd.  (Broadcast operands disable the DVE 2x mode on real HW, and
gpsimd cannot touch PSUM - both constrain this split.)

All heavy streams are bf16 and shaped [128, *].

Sharding: edges are contiguous, 25000 per core; angles are binned by owner
core (kj // 25000).  No collective is needed.
"""

import numpy as np
import ml_dtypes

import concourse.bass as bass
import concourse.mybir as mybir
import concourse.tile as tile
from concourse import bacc, bass_utils, library_config

F32 = mybir.dt.float32
BF16 = mybir.dt.bfloat16
AF = mybir.ActivationFunctionType
OP = mybir.AluOpType
BF = ml_dtypes.bfloat16

D = 128
A_DIM = 49
N_RBF = 6
N_BIL = 8
N_CORES = 8

# tuning knobs
PSUM_BF16 = False         # matmul outputs must be fp32 (hw restriction)
MUL_PATTERN = "DDCDBD"    # per-group scale-mul engine rotation
                          # (D=DVE direct, C=8 per-j ACT scaled copies,
                          #  B=ACT copy to sbuf + gpsimd mul)
L23_GP = True             # last two j-reduction levels on gpsimd (no PSUM
                          # access there, but z is SBUF bf16)
L1_GP_EVERY = 0           # chunks k%N==0 run the L1 add on gpsimd too
LAG = 3                   # apply trails the S-chunk loop by this many chunks


class Cfg:
    def __init__(self, e_valid, e_pad, r_list):
        self.e_valid = e_valid
        self.e_pad = e_pad
        self.r_list = tuple(r_list)
        assert e_pad % 2048 == 0
        assert len(self.r_list) == e_pad // 512
        for r in self.r_list:
            assert r % 2 == 0 and r >= 2
        self.xg = e_pad // 128            # groups of 128 edges
        self.n_chunks_b = e_pad // 1024   # me/apply chunks
        self.n_blocks = sum(r // 2 for r in self.r_list)
        self.at_cols = ((self.n_blocks * 512 + 2047) // 2048) * 2048
        # column base (in 512-blocks) of each chunk's rank stream; chunks are
        # processed lightest-first (reversed), so bases follow that order
        self.blk_base = [0] * len(self.r_list)
        base = 0
        for c in reversed(range(len(self.r_list))):
            self.blk_base[c] = base
            base += self.r_list[c] // 2

    def key(self):
        return (self.e_valid, self.e_pad, self.r_list)


def build_nc(cfg: Cfg):
    nc = bacc.Bacc(None)
    EP = cfg.e_pad
    XG = cfg.xg
    NCH = EP // 512
    YDT = BF16 if PSUM_BF16 else F32

    aT = nc.dram_tensor("a_t", [128, cfg.at_cols], BF16, kind="ExternalInput")
    mjiT = nc.dram_tensor("mji_t", [D, EP], BF16, kind="ExternalInput")
    erbf = nc.dram_tensor("erbf_t", [N_RBF, EP], BF16, kind="ExternalInput")
    wnbr = nc.dram_tensor("w_nbr", [D, D], BF16, kind="ExternalInput")
    bnbr = nc.dram_tensor("b_nbr", [D, 1], F32, kind="ExternalInput")
    wes = nc.dram_tensor("w_e", [N_RBF, D], BF16, kind="ExternalInput")
    wa2 = nc.dram_tensor("w_a2", [128, N_BIL], BF16, kind="ExternalInput")
    i8d = nc.dram_tensor("i8", [N_BIL, N_BIL], BF16, kind="ExternalInput")
    # j-outer: t2[l, j*128+i] = final_w[i, j, l]
    t2 = nc.dram_tensor("t2", [D, N_BIL * D], BF16, kind="ExternalInput")
    # partition-major output: outd[p, g*128 + i] = final[slot g*128 + p, i]
    outd = nc.dram_tensor("out", [128, EP], BF16, kind="ExternalOutput")

    with tile.TileContext(nc) as tc:
        nc.gpsimd.load_library(library_config.standard)
        with tc.tile_pool(name="const", bufs=1) as cp:
            wa_sb = cp.tile([128, N_BIL], BF16)
            nc.sync.dma_start(out=wa_sb[:], in_=wa2[:])
            i8_sb = cp.tile([N_BIL, N_BIL], BF16)
            nc.sync.dma_start(out=i8_sb[:], in_=i8d[:])
            wn_sb = cp.tile([D, D], BF16)
            nc.sync.dma_start(out=wn_sb[:], in_=wnbr[:])
            b_sb = cp.tile([D, 1], F32)
            nc.sync.dma_start(out=b_sb[:], in_=bnbr[:])
            we_sb = cp.tile([N_RBF, D], BF16)
            nc.sync.dma_start(out=we_sb[:], in_=wes[:])
            t2_sb = cp.tile([D, N_BIL * D], BF16)
            nc.sync.dma_start(out=t2_sb[:], in_=t2[:])
            s_f32 = cp.tile([128, XG * N_BIL], F32)   # S, group-major
            me_sb = cp.tile([128, EP], BF16)          # edge features

            with tc.tile_pool(name="pa", bufs=6) as pa, \
                 tc.tile_pool(name="stp", bufs=3) as stp, \
                 tc.tile_pool(name="pbm", bufs=6) as pbm, \
                 tc.tile_pool(name="zp", bufs=3) as zp, \
                 tc.tile_pool(name="yp", bufs=3) as yp, \
                 tc.tile_pool(name="osp", bufs=4) as osp, \
                 tc.tile_pool(name="pss", bufs=1, space="PSUM") as pss, \
                 tc.tile_pool(name="pst", bufs=1, space="PSUM") as pst, \
                 tc.tile_pool(name="pw", bufs=3, space="PSUM") as pw:

                at_tiles = {}

                def at_block(b):
                    ck = b // 4
                    if ck not in at_tiles:
                        t = pa.tile([128, 2048], BF16, tag="at")
                        nc.sync.dma_start(
                            out=t[:], in_=aT[:, ck * 2048:(ck + 1) * 2048])
                        at_tiles.clear()
                        at_tiles[ck] = t
                    off = (b % 4) * 512
                    return at_tiles[ck][:, off:off + 512]

                def emit_me_chunk(c):
                    er_t = pbm.tile([N_RBF, 1024], BF16, tag="er")
                    nc.sync.dma_start(out=er_t[:],
                                      in_=erbf[:, c * 1024:(c + 1) * 1024])
                    te_ps = pw.tile([128, 1024], YDT, tag="w")
                    for n in range(2):
                        nc.tensor.matmul(
                            te_ps[:, n * 512:(n + 1) * 512],
                            we_sb[:], er_t[:, n * 512:(n + 1) * 512],
                            start=True, stop=True)
                    mj = pbm.tile([128, 1024], BF16, tag="mj")
                    nc.sync.dma_start(out=mj[:],
                                      in_=mjiT[:, c * 1024:(c + 1) * 1024])
                    tm_ps = pw.tile([128, 1024], YDT, tag="w")
                    for n in range(2):
                        nc.tensor.matmul(
                            tm_ps[:, n * 512:(n + 1) * 512],
                            wn_sb[:], mj[:, n * 512:(n + 1) * 512],
                            start=True, stop=True)
                    tm_sb = pbm.tile([128, 1024], BF16, tag="tm_sb")
                    nc.scalar.activation(tm_sb[:], tm_ps[:], AF.Silu,
                                         bias=b_sb[:, 0:1])
                    te_sb = pbm.tile([128, 1024], BF16, tag="te_sb")
                    nc.scalar.activation(te_sb[:], te_ps[:], AF.Copy)
                    nc.vector.tensor_mul(
                        me_sb[:, c * 1024:(c + 1) * 1024], tm_sb[:], te_sb[:])

                pt_state = [None]
                cur_w = [-1]

                def emit_s_chunk(c):
                    # rank-pair matmuls + transpose for one 512-edge chunk
                    R = cfg.r_list[c]
                    blk = cfg.blk_base[c]
                    ps = pss.tile([N_BIL, 512], F32, tag="s")
                    for p in range(R // 2):
                        nc.tensor.matmul(
                            ps[:], wa_sb[:], at_block(blk + p),
                            start=(p == 0), stop=(p == R // 2 - 1))
                    st = stp.tile([N_BIL, 512], BF16, tag="st")
                    nc.scalar.activation(st[:], ps[:], AF.Copy)
                    for q in range(4):
                        g = c * 4 + q
                        slot = g % 64
                        if g // 64 != cur_w[0]:
                            pt_state[0] = pst.tile([128, 512], F32, tag="tp",
                                                   name="pt")
                            cur_w[0] = g // 64
                        pt = pt_state[0]
                        nc.tensor.matmul(
                            pt[:, slot * 8:(slot + 1) * 8],
                            st[:, q * 128:(q + 1) * 128],
                            i8_sb[:], start=True, stop=True)
                    # chunks run in descending order: after an even chunk,
                    # groups c*4 .. c*4+7 (chunks c, c+1) are complete
                    if c % 2 == 0:
                        g0 = c * 4
                        b0 = g0 % 64
                        nc.scalar.activation(
                            s_f32[:, g0 * 8:(g0 + 8) * 8],
                            pt[:, b0 * 8:(b0 + 8) * 8], AF.Copy)

                mul_cnt = [0]
                z_tiles = {}

                def emit_apply_muls(k, half):
                    # y = me_g^T @ t2 (j-outer), z = y * S
                    if half == 0:
                        z_tiles[k] = zp.tile([128, 8192], BF16, tag="z", name="z")
                    z = z_tiles[k]
                    for tt in range(half * 4, half * 4 + 4):
                        g = k * 8 + tt
                        y = pw.tile([128, 1024], YDT, tag="w")
                        lhsT = me_sb[:, g * 128:(g + 1) * 128]
                        nc.tensor.matmul(y[:, 0:512], lhsT, t2_sb[:, 0:512],
                                         start=True, stop=True)
                        nc.tensor.matmul(y[:, 512:1024], lhsT,
                                         t2_sb[:, 512:1024],
                                         start=True, stop=True)
                        sv = (s_f32[:, g * 8:(g + 1) * 8]
                              .to_broadcast((128, N_BIL, D)))
                        zt = z[:, tt * 1024:(tt + 1) * 1024]
                        zs = zt.rearrange("p (j i) -> p j i", j=N_BIL)
                        eng = MUL_PATTERN[mul_cnt[0] % len(MUL_PATTERN)]
                        mul_cnt[0] += 1
                        yv = y[:].rearrange("p (j i) -> p j i", j=N_BIL)
                        if eng == "B":
                            ysb = yp.tile([128, 1024], BF16, tag="y")
                            nc.scalar.activation(ysb[:], y[:], AF.Copy)
                            nc.gpsimd.tensor_mul(
                                zs, ysb[:].rearrange("p (j i) -> p j i",
                                                     j=N_BIL), sv)
                        elif eng == "C":
                            for j in range(N_BIL):
                                nc.scalar.activation(
                                    zt[:, j * D:(j + 1) * D],
                                    y[:, j * D:(j + 1) * D], AF.Copy,
                                    scale=s_f32[:, g * 8 + j:g * 8 + j + 1])
                        else:
                            nc.vector.tensor_mul(zs, yv, sv)

                def emit_apply_adds(k):
                    # batched j-reduction (tree): two 2x levels + final
                    z = z_tiles.pop(k)
                    zv = z[:].rearrange("p (t j i) -> p t j i", t=8,
                                        j=N_BIL)
                    tail = k == 0
                    eng1 = nc.gpsimd if (L1_GP_EVERY and
                                         k % L1_GP_EVERY == 0) else nc.vector
                    eng1.tensor_add(zv[:, :, 0:4, :], zv[:, :, 0:4, :],
                                    zv[:, :, 4:8, :])
                    eng2 = nc.gpsimd if (L23_GP and not tail) else nc.vector
                    eng2.tensor_add(zv[:, :, 0:2, :], zv[:, :, 0:2, :],
                                    zv[:, :, 2:4, :])
                    out_sb = osp.tile([128, 1024], BF16, tag="os")
                    eng2.tensor_add(
                        out_sb[:].rearrange("p (t i) -> p t i", t=8),
                        zv[:, :, 0, :], zv[:, :, 1, :])
                    nc.sync.dma_start(
                        out=outd[:, k * 1024:(k + 1) * 1024], in_=out_sb[:])

                NCB = cfg.n_chunks_b
                for m in range(4):
                    emit_me_chunk(NCB - 1 - m)
                me_done = 4
                next_half = 0         # apply half-chunks emitted
                adds_done = 0
                NH = 2 * NCB
                for i, c in enumerate(reversed(range(NCH))):
                    emit_s_chunk(c)
                    if i % 2 == 1 and me_done < NCB:
                        emit_me_chunk(NCB - 1 - me_done)
                        me_done += 1
                    while (next_half < NH
                           and next_half <= i - LAG):
                        emit_apply_muls(NCB - 1 - next_half // 2,
                                        next_half % 2)
                        next_half += 1
                        if adds_done < next_half // 2 - 1:
                            emit_apply_adds(NCB - 1 - adds_done)
                            adds_done += 1
                while me_done < NCB:
                    emit_me_chunk(NCB - 1 - me_done)
                    me_done += 1
                while next_half < NH:
                    emit_apply_muls(NCB - 1 - next_half // 2, next_half % 2)
                    next_half += 1
                    if adds_done < next_half // 2 - 1:
                        emit_apply_adds(NCB - 1 - adds_done)
                        adds_done += 1
                while adds_done < NCB:
                    emit_apply_adds(NCB - 1 - adds_done)
                    adds_done += 1
    nc.finalize()
    return nc


# ----------------------------------------------------------------------------
# host-side sharding / unsharding
# ----------------------------------------------------------------------------

def make_cfg(kj, n_edges, ev=25_000, ep=26_624):
    n_cores = (n_edges + ev - 1) // ev
    owner = np.minimum(kj // ev, n_cores - 1)
    nch = ep // 512
    r_list = np.full(nch, 2, np.int64)
    for c in range(n_cores):
        loc = kj[owner == c] - c * ev
        cnt = np.bincount(loc, minlength=ep)
        scnt = np.sort(cnt)[::-1]
        # chunk max = first (largest) count in each 512-slot chunk
        cmax = scnt[::512]
        r = np.maximum(2, ((cmax + 1) // 2) * 2)
        r_list = np.maximum(r_list, r)
    return Cfg(ev, ep, r_list.tolist())


def prep_in_maps(cfg: Cfg, m_ji, nbr_list, angle_list, e_rbf, a_sbf, kj_idx,
                 W_nbr, b_nbr, W_e, W_a, final_w):
    del nbr_list, angle_list
    m_ji = np.asarray(m_ji, np.float32)
    e_rbf = np.asarray(e_rbf, np.float32)
    a_sbf = np.asarray(a_sbf, np.float32)
    kj = np.asarray(kj_idx).astype(np.int64)
    W_nbr = np.asarray(W_nbr, np.float32)
    b_nbr = np.asarray(b_nbr, np.float32)
    W_e = np.asarray(W_e, np.float32)
    W_a = np.asarray(W_a, np.float32)
    final_w = np.asarray(final_w, np.float32)

    n_edges = m_ji.shape[0]
    ev = cfg.e_valid
    ep = cfg.e_pad
    n_cores = (n_edges + ev - 1) // ev
    owner = np.minimum(kj // ev, n_cores - 1)

    wa2 = np.zeros((128, N_BIL), np.float32)
    wa2[0:A_DIM] = W_a
    wa2[64:64 + A_DIM] = W_a
    # j-outer layout: t2[l, j*128+i] = final_w[i, j, l]
    t2 = np.ascontiguousarray(
        final_w.transpose(2, 1, 0).reshape(D, N_BIL * D))
    bn = np.ascontiguousarray(b_nbr.reshape(D, 1))
    i8 = np.eye(N_BIL, dtype=np.float32)

    in_maps = []
    orders = []
    for c in range(n_cores):
        sel = np.nonzero(owner == c)[0]
        loc = kj[sel] - c * ev
        aorder = np.argsort(loc, kind="stable")
        loc_s = loc[aorder]
        rows = sel[aorder]                      # a_sbf row per sorted token
        cnt = np.bincount(loc, minlength=ep)
        starts = np.concatenate([[0], np.cumsum(cnt)])

        eorder = np.argsort(-cnt, kind="stable")   # slot -> local edge
        inv = np.empty(ep, np.int64)
        inv[eorder] = np.arange(ep)                # local edge -> slot
        orders.append(eorder)

        # token -> (slot, rank) -> aT column/row-half
        n_tok = len(rows)
        le_of_t = np.repeat(np.arange(ep), cnt)
        r_of_t = np.arange(n_tok) - starts[le_of_t]
        slot_t = inv[le_of_t]
        ch_t = slot_t // 512
        assert (r_of_t < np.asarray(cfg.r_list)[ch_t]).all()
        col_t = (np.asarray(cfg.blk_base)[ch_t] + r_of_t // 2) * 512 \
            + slot_t % 512
        odd_t = (r_of_t % 2).astype(bool)

        at = np.zeros((128, cfg.at_cols), BF)
        vals = a_sbf[rows].astype(BF)              # [n_tok, 49]
        at[0:A_DIM, col_t[~odd_t]] = vals[~odd_t].T
        at[64:64 + A_DIM, col_t[odd_t]] = vals[odd_t].T

        e0, e1 = c * ev, min((c + 1) * ev, n_edges)
        valid = eorder < (e1 - e0)
        mjiT = np.zeros((D, ep), np.float32)
        mjiT[:, valid] = m_ji[e0:e1][eorder[valid]].T
        erbfT = np.zeros((N_RBF, ep), np.float32)
        erbfT[:, valid] = e_rbf[e0:e1][eorder[valid]].T

        im = {
            "a_t": at, "mji_t": mjiT.astype(BF),
            "erbf_t": erbfT.astype(BF), "w_nbr": W_nbr.astype(BF),
            "b_nbr": bn, "w_e": W_e.astype(BF), "w_a2": wa2.astype(BF),
            "i8": i8.astype(BF), "t2": t2.astype(BF),
        }
        in_maps.append(im)
    return in_maps, orders


def gather_output(cfg: Cfg, results, orders, n_edges):
    outs = []
    ev = cfg.e_valid
    for c, r in enumerate(results):
        e0, e1 = c * ev, min((c + 1) * ev, n_edges)
        o = np.asarray(r["out"]).astype(np.float32)      # [128, EP]
        o = o.reshape(128, cfg.xg, D).transpose(1, 0, 2).reshape(cfg.e_pad, D)
        inv = np.empty(cfg.e_pad, np.int64)
        inv[orders[c]] = np.arange(cfg.e_pad)
        outs.append(o[inv[:e1 - e0]])
    return np.ascontiguousarray(np.concatenate(outs, axis=0))


_NC_CACHE = {}


def run_on_hw(inputs, cfg=None, trace=False, trace_cores=None):
    kj = np.asarray(inputs["kj_idx"]).astype(np.int64)
    if cfg is None:
        cfg = make_cfg(kj, inputs["m_ji"].shape[0])
    key = cfg.key()
    if key not in _NC_CACHE:
        _NC_CACHE[key] = build_nc(cfg)
    nc = _NC_CACHE[key]
    in_maps, orders = prep_in_maps(cfg, **inputs)
    res = bass_utils.run_bass_kernel_spmd(
        nc, in_maps, core_ids=list(range(len(in_maps))),
        trace=trace, trace_cores=trace_cores)
    out = gather_output(cfg, res.results, orders, inputs["m_ji"].shape[0])
    return out, res


def kernel(**inputs) -> np.ndarray:
    out, _ = run_on_hw(inputs)
    return out
